# revision 25
# baseline (speedup 1.0000x reference)
"""Expected Calibration Error kernel for 8 Trainium2 NeuronCores.

Design (v2, "quantized-grid tilt-pack"):
  - Pad N=1,000,000 rows to 1,007,616 = 8 * 123 * 1024 with all-zero logit
    rows (label 55); each core processes 123 tiles of 1024 rows
    ([128 partitions x (8 rows * 100 classes)]).
  - Per tile:
      ACT:  t = l*1024 + 1.5*2^23      (magic add: rounds l*1024 to int)
      DVE:  r = t - 1.5*2^23           (= round(l*1024), integer f32)
      ACT:  e = exp(t*2^-10 - 12288) -> bf16   (= exp(r/1024), quantized-
            logit softmax numerator; |l|<8 so no overflow)
      k-chain (argmax+max packed in one value, delta = 2^-10):
        kt   = r + iota100*delta                (GPSIMD tensor_tensor add)
        kmax = segmented max(kt)                (DVE tensor_reduce)
      => kmax = r_max + delta*argmax EXACTLY (r integer, tilt < 0.5, f32
         grid arithmetic exact for |r| < 2^13).
      S-chain: bf16 pairwise adds (2x DVE mode) + f32 reduce:
        sh1 = e_lo50 + e_hi50; sh2 = sh1_lo25 + sh1_hi25; S = sum(sh2)
      The per-tile reduces are issued one tile late (software pipelining)
      so the in-order DVE never head-blocks on GPSIMD/ACT producers.
  - Epilogue (batched over 984 staged columns):
      lq = round(kmax) via magic; c* = (kmax - lq)*1024 exact;
      acc = (kmax - lq == label*delta); conf = exp(lq*2^-10) * (1/S);
      bin = round(15*conf - 0.5) via magic -> bf16;
      one-hot per bin via 15 tensor_scalar is_eq (bf16, 4x mode);
      PE matmuls accumulate per-bin (count, conf, acc) into PSUM [120, 24]
      (rows b*8+j; valid blocks on the j==j' diagonal).
  - Host: sum diagonal blocks over cores/j, subtract pad-row contribution
    (bin 0, conf 1/100, acc 0), finish the 15-bin ECE reduction.
"""

import sys

sys.path.insert(0, "/opt/trn_rl_repo")

import numpy as np

import concourse.bass as bass
import concourse.mybir as mybir
import concourse.tile as tile
from concourse.vector_clock import ScopedClock

F32 = mybir.dt.float32
I32 = mybir.dt.int32
BF16 = mybir.dt.bfloat16

N_BINS = 15
C = 100
ROWS_PER_TILE = 1024
A = 8  # rows per partition per tile
N_FULL = 1_000_000
N_CORES = 8
TILES = 123
ROWS_PER_CORE = TILES * ROWS_PER_TILE  # 125952
N_PAD = N_CORES * ROWS_PER_CORE       # 1007616
PAD_LABEL = 55
GROUPS_PER_MM = 8

MAG = 12582912.0          # 1.5 * 2^23: f32 magic for round-to-int
SC = 1024.0               # logit quantization scale (grid 2^-10)
DELTA = 1.0 / 1024.0      # class-index tilt unit (99*DELTA < 0.5)


def _patch_tile_drain():
    """walrus in this container allows only 1 sync wait per instruction; split
    every multi-wait instruction's extra waits onto preceding same-engine
    no-ops, and the TileContext exit drain's waits across a chain of drains."""
    if getattr(tile.TileContext, "_drain_patched", False):
        return

    orig_lower = tile.TileContext._lower_ordered_insts

    def _lower_ordered_insts(self, ordered):
        for insts in ordered.values():
            if not isinstance(insts, list):
                continue
            new = []
            for inst in insts:
                si = getattr(inst, "sync_info", None)
                waits = list(si.on_wait) if si is not None else []
                if len(waits) > 1 and isinstance(inst, mybir.Instruction):
                    si.on_wait = waits[-1:]
                    for j, w in enumerate(waits[:-1]):
                        nop = mybir.InstNoOp(
                            name=f"{inst.name}-ws{j}",
                            sync_info=mybir.SyncInfo(on_wait=[w], on_update=[]),
                            bass_nofuse=True,
                            engine=inst.engine,
                        )
                        new.append(nop)
                new.append(inst)
            insts[:] = new
        return orig_lower(self, ordered)

    tile.TileContext._lower_ordered_insts = _lower_ordered_insts

    orig_commit = tile.TileContext._commit_instruction

    def _commit_instruction(self, inst, *args, **kwargs):
        si = getattr(inst, "sync_info", None)
        if si is not None and len(si.on_wait) > 1 and isinstance(inst, mybir.Instruction):
            waits = list(si.on_wait)
            si.on_wait = waits[-1:]
            for j, w in enumerate(waits[:-1]):
                nop = mybir.InstNoOp(
                    name=f"{inst.name}-cs{j}",
                    sync_info=mybir.SyncInfo(on_wait=[w], on_update=[]),
                    bass_nofuse=True,
                    engine=inst.engine,
                )
                orig_commit(self, nop)
        return orig_commit(self, inst, *args, **kwargs)

    tile.TileContext._commit_instruction = _commit_instruction

    def _drain_and_barrier(self, tick_clock, wait_clock):
        drain_inst = self.nc.sync.drain()
        wait_clock.add_sem_waits(
            drain_inst.ins, ScopedClock({None: tick_clock.global_clock})
        )
        si = drain_inst.ins.sync_info
        waits = list(si.on_wait) if si else []
        if len(waits) > 1:
            si.on_wait = waits[:1]
            for i in range(1, len(waits)):
                d2 = self.nc.sync.drain()
                d2.ins.sync_info = type(si)(on_wait=waits[i : i + 1], on_update=[])
        self.nc.all_engine_barrier()
        popped = self.nc._tile_sem_poison_stack.pop()
        assert popped is self._sem_poison
        self.nc.clear_and_free_semaphores(list(self.sems.allocated().values()))
        self.nc.all_engine_barrier()

    tile.TileContext._drain_and_barrier = _drain_and_barrier
    tile.TileContext._drain_patched = True


def build_nc(tiles=TILES, repeat=1, ablate="", io_bufs=8, ek_bufs=4,
             kh_eng="dve", kt_eng="gpsimd", klevel=0, r_act_m=0):
    """ablate letters: 'q' quantize, 'k' k-chain, 'e' exp, 's' S-chain,
    'E' epilogue, 'D' per-tile input DMA.
    klevel: 0 = flat tilt-add (kt_eng) + full-width max-reduce;
            1 = one halving level (kh_eng max) before tilt+reduce."""
    _patch_tile_drain()
    cols = tiles * A
    n_groups = cols // GROUPS_PER_MM
    assert cols % GROUPS_PER_MM == 0

    nc = bass.Bass(trn_type="TRN2")
    lg = nc.declare_dram_parameter("lg", [tiles * ROWS_PER_TILE, C], F32, isOutput=False)
    lb = nc.declare_dram_parameter("lb", [128, cols], I32, isOutput=False)
    part = nc.declare_dram_parameter("part", [GROUPS_PER_MM * N_BINS, GROUPS_PER_MM * 3], F32, isOutput=True)

    khe = getattr(nc, {"gpsimd": "gpsimd", "dve": "vector"}[kh_eng])
    kte = getattr(nc, {"gpsimd": "gpsimd", "dve": "vector"}[kt_eng])

    with tile.TileContext(nc) as tc:
        with (
            tc.tile_pool(name="io", bufs=io_bufs) as io_pool,
            tc.tile_pool(name="ek", bufs=ek_bufs) as ek_pool,
            tc.tile_pool(name="stage", bufs=1) as stage,
            tc.tile_pool(name="psum", bufs=1, space="PSUM") as psum_pool,
        ):
            # ---- constants ----
            # tilt tiles: iota * DELTA over the within-segment class index
            tw = 50 if klevel == 1 else C  # tilt width per segment
            it_i = stage.tile([128, A * tw], I32, tag="it_i")
            nc.gpsimd.iota(it_i[:], pattern=[[0, A], [1, tw]], base=0, channel_multiplier=0)
            iotad = stage.tile([128, A * tw], F32, tag="iotad")
            nc.vector.tensor_copy(iotad[:], it_i[:])
            nc.vector.tensor_scalar_mul(iotad[:], iotad[:], DELTA)

            # per-partition scalar bias tiles for ACT
            bias_mag = stage.tile([128, 1], F32, tag="bias_mag")
            nc.vector.memset(bias_mag[:], MAG)
            bias_m12288 = stage.tile([128, 1], F32, tag="bias_m12288")
            nc.vector.memset(bias_m12288[:], -12288.0)

            # ---- staging ----
            kmax_all = stage.tile([128, cols], F32, tag="kmax_all")
            s_all = stage.tile([128, cols], F32, tag="s_all")
            lab_all = stage.tile([128, cols], I32, tag="lab_all")
            if ablate:
                nc.vector.memset(kmax_all[:], 0.0)
                nc.vector.memset(s_all[:], 1.0)

            # labels: one contiguous DMA (host pre-transposed layout)
            nc.sync.dma_start(out=lab_all[:], in_=lb[:, :])
            labd = stage.tile([128, cols], F32, tag="labd")
            nc.vector.tensor_copy(labd[:], lab_all[:])
            nc.vector.tensor_scalar_mul(labd[:], labd[:], DELTA)

            if "D" in ablate:
                fixed_tile = stage.tile([128, A * C], F32, tag="fixed_tile")
                nc.sync.dma_start(
                    out=fixed_tile[:],
                    in_=lg[0:ROWS_PER_TILE, :].rearrange("(p a) c -> p (a c)", p=128),
                )

            import contextlib

            loop_cm = tc.For_i(0, repeat, 1) if repeat > 1 else contextlib.nullcontext()

            def consume(t, kt_t, e_t):
                """reduces for tile t (issued one tile late: producers done)."""
                if kt_t is not None:
                    h = C if klevel == 0 else 50
                    nc.vector.tensor_reduce(
                        out=kmax_all[:, t * A : (t + 1) * A],
                        in_=kt_t[:].rearrange("p (a h) -> p a h", h=h),
                        axis=mybir.AxisListType.X,
                        op=mybir.AluOpType.max,
                    )
                if e_t is not None:
                    e3 = e_t[:].rearrange("p (a c) -> p a c", c=C)
                    sh1 = ek_pool.tile([128, A * 50], BF16, tag="sh1")
                    sh13 = sh1[:].rearrange("p (a h) -> p a h", h=50)
                    nc.vector.tensor_tensor(out=sh13, in0=e3[:, :, 0:50], in1=e3[:, :, 50:100], op=mybir.AluOpType.add)
                    sh2 = ek_pool.tile([128, A * 25], BF16, tag="sh2")
                    sh23 = sh2[:].rearrange("p (a h) -> p a h", h=25)
                    nc.vector.tensor_tensor(out=sh23, in0=sh13[:, :, 0:25], in1=sh13[:, :, 25:50], op=mybir.AluOpType.add)
                    nc.vector.tensor_reduce(
                        out=s_all[:, t * A : (t + 1) * A],
                        in_=sh23,
                        axis=mybir.AxisListType.X,
                        op=mybir.AluOpType.add,
                    )

            with loop_cm:
                pending = None  # (t, kt_tile, e_tile) awaiting reduces
                for t in range(tiles):
                    if "D" in ablate:
                        l_tile = fixed_tile
                    else:
                        l_tile = io_pool.tile([128, A * C], F32, tag="l_tile")
                        r0 = t * ROWS_PER_TILE
                        nc.sync.dma_start(
                            out=l_tile[:],
                            in_=lg[r0 : r0 + ROWS_PER_TILE, :].rearrange(
                                "(p a) c -> p (a c)", p=128
                            ),
                        )

                    if "q" in ablate:
                        t_tile = l_tile
                        r_tile = l_tile
                    else:
                        t_tile = ek_pool.tile([128, A * C], F32, tag="t_tile")
                        nc.scalar.activation(
                            t_tile[:], l_tile[:],
                            mybir.ActivationFunctionType.Identity,
                            bias=bias_mag[:], scale=SC,
                        )
                        r_tile = ek_pool.tile([128, A * C], F32, tag="r_tile")
                        if r_act_m and t % r_act_m == r_act_m - 1:
                            # rebalance: ACT has slack, DVE is the wall
                            nc.scalar.activation(
                                r_tile[:], t_tile[:],
                                mybir.ActivationFunctionType.Copy,
                                bias=-MAG,
                            )
                        else:
                            nc.vector.tensor_scalar_sub(r_tile[:], t_tile[:], MAG)

                    e_tile = None
                    if "e" not in ablate:
                        e_tile = ek_pool.tile([128, A * C], BF16, tag="e_tile")
                        nc.scalar.activation(
                            e_tile[:], t_tile[:],
                            mybir.ActivationFunctionType.Exp,
                            bias=bias_m12288[:], scale=(1.0 / SC),
                        )

                    kt = None
                    if "k" not in ablate:
                        r3 = r_tile[:].rearrange("p (a c) -> p a c", c=C)
                        if klevel == 0:
                            # flat: kt = r + iota*DELTA (GP add)
                            kt = ek_pool.tile([128, A * C], F32, tag="kt")
                            kte.tensor_tensor(out=kt[:], in0=r_tile[:], in1=iotad[:], op=mybir.AluOpType.add)
                        else:
                            # one halving level: z1 = hi + 50d; kh1 = max(lo, z1)
                            z1 = ek_pool.tile([128, A * 50], F32, tag="z1")
                            z13 = z1[:].rearrange("p (a h) -> p a h", h=50)
                            nc.vector.tensor_scalar_add(z13, r3[:, :, 50:100], 50.0 * DELTA)
                            kh1 = ek_pool.tile([128, A * 50], F32, tag="kh1")
                            kh13 = kh1[:].rearrange("p (a h) -> p a h", h=50)
                            khe.tensor_tensor(out=kh13, in0=r3[:, :, 0:50], in1=z13, op=mybir.AluOpType.max)
                            kt = ek_pool.tile([128, A * 50], F32, tag="kt")
                            kte.tensor_tensor(out=kt[:], in0=kh1[:], in1=iotad[:], op=mybir.AluOpType.add)

                    et = e_tile if "s" not in ablate else None
                    if pending is not None:
                        consume(*pending)
                    pending = (t, kt, et)
                if pending is not None:
                    consume(*pending)
                    pending = None

            if "E" not in ablate:
                # ---- epilogue ----
                u = stage.tile([128, cols], F32, tag="u")
                nc.vector.tensor_scalar_add(u[:], kmax_all[:], MAG)
                lq = stage.tile([128, cols], F32, tag="lq")
                nc.vector.tensor_scalar_sub(lq[:], u[:], MAG)
                d = stage.tile([128, cols], F32, tag="d")
                nc.vector.tensor_sub(d[:], kmax_all[:], lq[:])
                acc = stage.tile([128, cols], F32, tag="acc")
                nc.vector.tensor_tensor(out=acc[:], in0=d[:], in1=labd[:], op=mybir.AluOpType.is_equal)

                em = stage.tile([128, cols], F32, tag="em")
                nc.scalar.activation(
                    em[:], lq[:], mybir.ActivationFunctionType.Exp, scale=(1.0 / SC)
                )
                rinv = stage.tile([128, cols], F32, tag="rinv")
                nc.vector.reciprocal(rinv[:], s_all[:])
                conf = stage.tile([128, cols], F32, tag="conf")
                nc.vector.tensor_mul(conf[:], em[:], rinv[:])

                y2 = stage.tile([128, cols], F32, tag="y2")
                nc.vector.tensor_scalar(
                    y2[:], conf[:], float(N_BINS), 0.5,
                    op0=mybir.AluOpType.mult, op1=mybir.AluOpType.subtract,
                )
                binb = stage.tile([128, cols], BF16, tag="binb")
                nc.vector.tensor_scalar(
                    binb[:], y2[:], MAG, MAG,
                    op0=mybir.AluOpType.add, op1=mybir.AluOpType.subtract,
                )

                # one-hot, group-contiguous layout: oh[p, g, b, j] so each
                # group's matmul lhsT slice is one contiguous 120-wide dim
                oh = stage.tile([128, N_BINS * cols], BF16, tag="oh")
                oh4 = oh[:].rearrange(
                    "p (g b j) -> p g b j", b=N_BINS, j=GROUPS_PER_MM
                )
                binb3 = binb[:].rearrange("p (g j) -> p g j", j=GROUPS_PER_MM)
                for b in range(N_BINS):
                    nc.vector.tensor_scalar(
                        oh4[:, :, b, :], binb3, float(b), None,
                        op0=mybir.AluOpType.is_equal,
                    )

                # V: interleaved (1, conf, acc) per staged column, bf16
                v_all = stage.tile([128, cols * 3], BF16, tag="v_all")
                nc.vector.memset(v_all[:], 1.0)
                v3 = v_all[:].rearrange("p (col three) -> p col three", three=3)
                nc.vector.tensor_copy(v3[:, :, 1], conf[:])
                nc.vector.tensor_copy(v3[:, :, 2], acc[:])

                # PE: accumulate per-bin partials over all column groups
                psum = psum_pool.tile([GROUPS_PER_MM * N_BINS, GROUPS_PER_MM * 3], F32)
                lw = N_BINS * GROUPS_PER_MM  # 120
                for g in range(n_groups):
                    nc.tensor.matmul(
                        out=psum[:],
                        lhsT=oh[:, g * lw : (g + 1) * lw],
                        rhs=v_all[:, g * GROUPS_PER_MM * 3 : (g + 1) * GROUPS_PER_MM * 3],
                        start=(g == 0),
                        stop=(g == n_groups - 1),
                    )

                out_sb = stage.tile([GROUPS_PER_MM * N_BINS, GROUPS_PER_MM * 3], F32, tag="out_sb")
                nc.vector.tensor_copy(out_sb[:], psum[:])
                nc.sync.dma_start(out=part[:], in_=out_sb[:])

    return nc


# ----------------------------------------------------------------------------
# host side
# ----------------------------------------------------------------------------

_RUNNER_CACHE = {}


def _get_runner(tiles=TILES, repeat=1, **opts):
    """Build (once) a jitted 8-core shard_map runner for the kernel."""
    key = (tiles, repeat, tuple(sorted(opts.items())))
    if key in _RUNNER_CACHE:
        return _RUNNER_CACHE[key]

    import jax
    from jax.sharding import Mesh, PartitionSpec
    try:
        from jax.experimental.shard_map import shard_map
    except ImportError:
        from jax.shard_map import shard_map
    from concourse import bass2jax

    nc = build_nc(tiles, repeat=repeat, **opts)
    bass2jax.install_neuronx_cc_hook()

    partition_name = nc.partition_id_tensor.name if nc.partition_id_tensor else None
    in_names = ["lg", "lb"]
    out_names = ["part"]
    out_avals = [jax.core.ShapedArray((GROUPS_PER_MM * N_BINS, GROUPS_PER_MM * 3), np.float32)]
    all_in_names = in_names + out_names + ([partition_name] if partition_name else [])

    def _body(*args):
        operands = list(args)
        if partition_name is not None:
            operands.append(bass2jax.partition_id_tensor())
        outs = bass2jax._bass_exec_p.bind(
            *operands,
            out_avals=tuple(out_avals),
            in_names=tuple(all_in_names),
            out_names=tuple(out_names),
            lowering_input_output_aliases=(),
            sim_require_finite=True,
            sim_require_nnan=True,
            nc=nc,
        )
        return tuple(outs)

    devices = jax.devices()[:N_CORES]
    mesh = Mesh(np.asarray(devices), ("core",))
    n_in = len(in_names) + len(out_avals)
    sharded = jax.jit(
        shard_map(
            _body,
            mesh=mesh,
            in_specs=(PartitionSpec("core"),) * n_in,
            out_specs=(PartitionSpec("core"),) * len(out_names),
            check_rep=False,
        ),
        donate_argnums=(len(in_names),),
        keep_unused=True,
    )
    _RUNNER_CACHE[key] = sharded
    return sharded


def _prep_inputs(logits, labels):
    logits = np.asarray(logits)
    labels = np.asarray(labels)
    n = logits.shape[0]
    assert logits.shape == (N_FULL, C) and n == N_FULL, logits.shape
    pad = N_PAD - n
    lg = np.concatenate([logits, np.zeros((pad, C), np.float32)], axis=0)
    lab32 = np.full(N_PAD, PAD_LABEL, np.int32)
    lab32[:n] = labels.astype(np.int32)
    # device layout: lab[core*128 + p, t*A + a] = label[core, t, p, a]
    lb = np.ascontiguousarray(
        lab32.reshape(N_CORES, TILES, 128, A).transpose(0, 2, 1, 3)
    ).reshape(N_CORES * 128, TILES * A)
    return lg, lb, pad


def _finish(parts, pad):
    """parts: (8, 120, 24) psum dumps -> ECE scalar (f32 [1]).

    psum row b*8+j holds bin-b sums for column-slot j; valid data are the
    j==j' diagonal blocks of the 8 (j') column triples."""
    agg = np.zeros((N_BINS, 3), np.float64)
    for c in range(parts.shape[0]):
        for j in range(GROUPS_PER_MM):
            for b in range(N_BINS):
                agg[b] += parts[c, b * GROUPS_PER_MM + j, j * 3 : (j + 1) * 3]
    counts = agg[:, 0].copy()
    conf_sums = agg[:, 1].copy()
    acc_sums = agg[:, 2].copy()
    # all-zero pad rows: conf = exp(0)/100 -> bin 0, acc = 0 (pred 99 != 55)
    counts[0] -= pad
    conf_sums[0] -= pad * float(np.float32(1.0) / np.float32(100.0))
    n = N_FULL
    prop = counts / n
    denom = np.maximum(counts, 1.0)
    avg_conf = conf_sums / denom
    avg_acc = acc_sums / denom
    per_bin = np.where(counts > 0, np.abs(avg_conf - avg_acc) * prop, 0.0)
    return np.array([per_bin.sum()], dtype=np.float32)


def kernel(logits, labels):
    lg, lb, pad = _prep_inputs(logits, labels)
    runner = _get_runner()
    zeros = np.zeros((N_CORES * GROUPS_PER_MM * N_BINS, GROUPS_PER_MM * 3), np.float32)
    last = None
    for attempt in range(3):
        try:
            (out,) = runner(lg, lb, np.zeros_like(zeros))
            parts = np.asarray(out).reshape(
                N_CORES, GROUPS_PER_MM * N_BINS, GROUPS_PER_MM * 3
            )
            return _finish(parts, pad)
        except Exception as e:  # transient NRT_EXEC_UNIT_UNRECOVERABLE etc.
            last = e
            import time as _time

            _time.sleep(20)
    raise last


# revision 26
# speedup vs baseline: 1.0382x; 1.0382x over previous
"""Expected Calibration Error kernel for 8 Trainium2 NeuronCores.

Design (v2, "quantized-grid tilt-pack"):
  - Pad N=1,000,000 rows to 1,007,616 = 8 * 123 * 1024 with all-zero logit
    rows (label 55); each core processes 123 tiles of 1024 rows
    ([128 partitions x (8 rows * 100 classes)]).
  - Per tile:
      ACT:  t = l*1024 + 1.5*2^23      (magic add: rounds l*1024 to int)
      DVE:  r = t - 1.5*2^23           (= round(l*1024), integer f32)
      ACT:  e = exp(t*2^-10 - 12288) -> bf16   (= exp(r/1024), quantized-
            logit softmax numerator; |l|<8 so no overflow)
      k-chain (argmax+max packed in one value, delta = 2^-10):
        kt   = r + iota100*delta                (GPSIMD tensor_tensor add)
        kmax = segmented max(kt)                (DVE tensor_reduce)
      => kmax = r_max + delta*argmax EXACTLY (r integer, tilt < 0.5, f32
         grid arithmetic exact for |r| < 2^13).
      S-chain: bf16 pairwise adds (2x DVE mode) + f32 reduce:
        sh1 = e_lo50 + e_hi50; sh2 = sh1_lo25 + sh1_hi25; S = sum(sh2)
      The per-tile reduces are issued one tile late (software pipelining)
      so the in-order DVE never head-blocks on GPSIMD/ACT producers.
  - Epilogue (batched over 984 staged columns):
      lq = round(kmax) via magic; c* = (kmax - lq)*1024 exact;
      acc = (kmax - lq == label*delta); conf = exp(lq*2^-10) * (1/S);
      bin = round(15*conf - 0.5) via magic -> bf16;
      one-hot per bin via 15 tensor_scalar is_eq (bf16, 4x mode);
      PE matmuls accumulate per-bin (count, conf, acc) into PSUM [120, 24]
      (rows b*8+j; valid blocks on the j==j' diagonal).
  - Host: sum diagonal blocks over cores/j, subtract pad-row contribution
    (bin 0, conf 1/100, acc 0), finish the 15-bin ECE reduction.
"""

import sys

sys.path.insert(0, "/opt/trn_rl_repo")

import numpy as np

import concourse.bass as bass
import concourse.mybir as mybir
import concourse.tile as tile
from concourse.vector_clock import ScopedClock

F32 = mybir.dt.float32
I32 = mybir.dt.int32
BF16 = mybir.dt.bfloat16

N_BINS = 15
C = 100
ROWS_PER_TILE = 1024
A = 8  # rows per partition per tile
N_FULL = 1_000_000
N_CORES = 8
TILES = 123
ROWS_PER_CORE = TILES * ROWS_PER_TILE  # 125952
N_PAD = N_CORES * ROWS_PER_CORE       # 1007616
PAD_LABEL = 55
GROUPS_PER_MM = 8

MAG = 12582912.0          # 1.5 * 2^23: f32 magic for round-to-int
SC = 1024.0               # logit quantization scale (grid 2^-10)
DELTA = 1.0 / 1024.0      # class-index tilt unit (99*DELTA < 0.5)


def _patch_tile_drain():
    """walrus in this container allows only 1 sync wait per instruction; split
    every multi-wait instruction's extra waits onto preceding same-engine
    no-ops, and the TileContext exit drain's waits across a chain of drains."""
    if getattr(tile.TileContext, "_drain_patched", False):
        return

    orig_lower = tile.TileContext._lower_ordered_insts

    def _lower_ordered_insts(self, ordered):
        for insts in ordered.values():
            if not isinstance(insts, list):
                continue
            new = []
            for inst in insts:
                si = getattr(inst, "sync_info", None)
                waits = list(si.on_wait) if si is not None else []
                if len(waits) > 1 and isinstance(inst, mybir.Instruction):
                    si.on_wait = waits[-1:]
                    for j, w in enumerate(waits[:-1]):
                        nop = mybir.InstNoOp(
                            name=f"{inst.name}-ws{j}",
                            sync_info=mybir.SyncInfo(on_wait=[w], on_update=[]),
                            bass_nofuse=True,
                            engine=inst.engine,
                        )
                        new.append(nop)
                new.append(inst)
            insts[:] = new
        return orig_lower(self, ordered)

    tile.TileContext._lower_ordered_insts = _lower_ordered_insts

    orig_commit = tile.TileContext._commit_instruction

    def _commit_instruction(self, inst, *args, **kwargs):
        si = getattr(inst, "sync_info", None)
        if si is not None and len(si.on_wait) > 1 and isinstance(inst, mybir.Instruction):
            waits = list(si.on_wait)
            si.on_wait = waits[-1:]
            for j, w in enumerate(waits[:-1]):
                nop = mybir.InstNoOp(
                    name=f"{inst.name}-cs{j}",
                    sync_info=mybir.SyncInfo(on_wait=[w], on_update=[]),
                    bass_nofuse=True,
                    engine=inst.engine,
                )
                orig_commit(self, nop)
        return orig_commit(self, inst, *args, **kwargs)

    tile.TileContext._commit_instruction = _commit_instruction

    def _drain_and_barrier(self, tick_clock, wait_clock):
        drain_inst = self.nc.sync.drain()
        wait_clock.add_sem_waits(
            drain_inst.ins, ScopedClock({None: tick_clock.global_clock})
        )
        si = drain_inst.ins.sync_info
        waits = list(si.on_wait) if si else []
        if len(waits) > 1:
            si.on_wait = waits[:1]
            for i in range(1, len(waits)):
                d2 = self.nc.sync.drain()
                d2.ins.sync_info = type(si)(on_wait=waits[i : i + 1], on_update=[])
        self.nc.all_engine_barrier()
        popped = self.nc._tile_sem_poison_stack.pop()
        assert popped is self._sem_poison
        self.nc.clear_and_free_semaphores(list(self.sems.allocated().values()))
        self.nc.all_engine_barrier()

    tile.TileContext._drain_and_barrier = _drain_and_barrier
    tile.TileContext._drain_patched = True


def build_nc(tiles=TILES, repeat=1, ablate="", io_bufs=8, ek_bufs=3,
             kh_eng="dve", kt_eng="gpsimd", klevel=0, r_act_m=0):
    """ablate letters: 'q' quantize, 'k' k-chain, 'e' exp, 's' S-chain,
    'E' epilogue, 'D' per-tile input DMA.
    klevel: 0 = flat tilt-add (kt_eng) + full-width max-reduce;
            1 = one halving level (kh_eng max) before tilt+reduce."""
    _patch_tile_drain()
    cols = tiles * A
    n_groups = cols // GROUPS_PER_MM
    assert cols % GROUPS_PER_MM == 0

    nc = bass.Bass(trn_type="TRN2")
    lg = nc.declare_dram_parameter("lg", [tiles * ROWS_PER_TILE, C], F32, isOutput=False)
    lb = nc.declare_dram_parameter("lb", [128, cols], I32, isOutput=False)
    part = nc.declare_dram_parameter("part", [GROUPS_PER_MM * N_BINS, GROUPS_PER_MM * 3], F32, isOutput=True)

    khe = getattr(nc, {"gpsimd": "gpsimd", "dve": "vector"}[kh_eng])
    kte = getattr(nc, {"gpsimd": "gpsimd", "dve": "vector"}[kt_eng])

    with tile.TileContext(nc) as tc:
        with (
            tc.tile_pool(name="io", bufs=io_bufs) as io_pool,
            tc.tile_pool(name="ek", bufs=ek_bufs) as ek_pool,
            tc.tile_pool(name="stage", bufs=1) as stage,
            tc.tile_pool(name="psum", bufs=1, space="PSUM") as psum_pool,
        ):
            # ---- constants ----
            # tilt tiles: iota * DELTA over the within-segment class index
            tw = 50 if klevel == 1 else C  # tilt width per segment
            it_i = stage.tile([128, A * tw], I32, tag="it_i")
            nc.gpsimd.iota(it_i[:], pattern=[[0, A], [1, tw]], base=0, channel_multiplier=0)
            iotad = stage.tile([128, A * tw], F32, tag="iotad")
            nc.vector.tensor_copy(iotad[:], it_i[:])
            nc.vector.tensor_scalar_mul(iotad[:], iotad[:], DELTA)

            # per-partition scalar bias tiles for ACT
            bias_mag = stage.tile([128, 1], F32, tag="bias_mag")
            nc.vector.memset(bias_mag[:], MAG)
            bias_m12288 = stage.tile([128, 1], F32, tag="bias_m12288")
            nc.vector.memset(bias_m12288[:], -12288.0)

            # ---- staging ----
            kmax_all = stage.tile([128, cols], F32, tag="kmax_all")
            s_all = stage.tile([128, cols], F32, tag="s_all")
            lab_all = stage.tile([128, cols], I32, tag="lab_all")
            if ablate:
                nc.vector.memset(kmax_all[:], 0.0)
                nc.vector.memset(s_all[:], 1.0)

            # labels: one contiguous DMA (host pre-transposed layout)
            nc.sync.dma_start(out=lab_all[:], in_=lb[:, :])
            labd = stage.tile([128, cols], F32, tag="labd")
            nc.vector.tensor_copy(labd[:], lab_all[:])
            nc.vector.tensor_scalar_mul(labd[:], labd[:], DELTA)

            if "D" in ablate:
                fixed_tile = stage.tile([128, A * C], F32, tag="fixed_tile")
                nc.sync.dma_start(
                    out=fixed_tile[:],
                    in_=lg[0:ROWS_PER_TILE, :].rearrange("(p a) c -> p (a c)", p=128),
                )

            import contextlib

            loop_cm = tc.For_i(0, repeat, 1) if repeat > 1 else contextlib.nullcontext()

            def consume(t, kt_t, e_t):
                """reduces for tile t (issued one tile late: producers done)."""
                if kt_t is not None:
                    h = C if klevel == 0 else 50
                    nc.vector.tensor_reduce(
                        out=kmax_all[:, t * A : (t + 1) * A],
                        in_=kt_t[:].rearrange("p (a h) -> p a h", h=h),
                        axis=mybir.AxisListType.X,
                        op=mybir.AluOpType.max,
                    )
                if e_t is not None:
                    e3 = e_t[:].rearrange("p (a c) -> p a c", c=C)
                    sh1 = ek_pool.tile([128, A * 50], BF16, tag="sh1")
                    sh13 = sh1[:].rearrange("p (a h) -> p a h", h=50)
                    nc.vector.tensor_tensor(out=sh13, in0=e3[:, :, 0:50], in1=e3[:, :, 50:100], op=mybir.AluOpType.add)
                    sh2 = ek_pool.tile([128, A * 25], BF16, tag="sh2")
                    sh23 = sh2[:].rearrange("p (a h) -> p a h", h=25)
                    nc.vector.tensor_tensor(out=sh23, in0=sh13[:, :, 0:25], in1=sh13[:, :, 25:50], op=mybir.AluOpType.add)
                    nc.vector.tensor_reduce(
                        out=s_all[:, t * A : (t + 1) * A],
                        in_=sh23,
                        axis=mybir.AxisListType.X,
                        op=mybir.AluOpType.add,
                    )

            with loop_cm:
                pending = None  # (t, kt_tile, e_tile) awaiting reduces
                for t in range(tiles):
                    if "D" in ablate:
                        l_tile = fixed_tile
                    else:
                        l_tile = io_pool.tile([128, A * C], F32, tag="l_tile")
                        r0 = t * ROWS_PER_TILE
                        nc.sync.dma_start(
                            out=l_tile[:],
                            in_=lg[r0 : r0 + ROWS_PER_TILE, :].rearrange(
                                "(p a) c -> p (a c)", p=128
                            ),
                        )

                    if "q" in ablate:
                        t_tile = l_tile
                        r_tile = l_tile
                    else:
                        t_tile = ek_pool.tile([128, A * C], F32, tag="t_tile")
                        nc.scalar.activation(
                            t_tile[:], l_tile[:],
                            mybir.ActivationFunctionType.Identity,
                            bias=bias_mag[:], scale=SC,
                        )
                        r_tile = ek_pool.tile([128, A * C], F32, tag="r_tile")
                        if r_act_m and t % r_act_m == r_act_m - 1:
                            # rebalance: ACT has slack, DVE is the wall
                            nc.scalar.activation(
                                r_tile[:], t_tile[:],
                                mybir.ActivationFunctionType.Copy,
                                bias=-MAG,
                            )
                        else:
                            nc.vector.tensor_scalar_sub(r_tile[:], t_tile[:], MAG)

                    e_tile = None
                    if "e" not in ablate:
                        e_tile = ek_pool.tile([128, A * C], BF16, tag="e_tile")
                        nc.scalar.activation(
                            e_tile[:], t_tile[:],
                            mybir.ActivationFunctionType.Exp,
                            bias=bias_m12288[:], scale=(1.0 / SC),
                        )

                    kt = None
                    if "k" not in ablate:
                        r3 = r_tile[:].rearrange("p (a c) -> p a c", c=C)
                        if klevel == 0:
                            # flat: kt = r + iota*DELTA (GP add)
                            kt = ek_pool.tile([128, A * C], F32, tag="kt")
                            kte.tensor_tensor(out=kt[:], in0=r_tile[:], in1=iotad[:], op=mybir.AluOpType.add)
                        else:
                            # one halving level: z1 = hi + 50d; kh1 = max(lo, z1)
                            z1 = ek_pool.tile([128, A * 50], F32, tag="z1")
                            z13 = z1[:].rearrange("p (a h) -> p a h", h=50)
                            nc.vector.tensor_scalar_add(z13, r3[:, :, 50:100], 50.0 * DELTA)
                            kh1 = ek_pool.tile([128, A * 50], F32, tag="kh1")
                            kh13 = kh1[:].rearrange("p (a h) -> p a h", h=50)
                            khe.tensor_tensor(out=kh13, in0=r3[:, :, 0:50], in1=z13, op=mybir.AluOpType.max)
                            kt = ek_pool.tile([128, A * 50], F32, tag="kt")
                            kte.tensor_tensor(out=kt[:], in0=kh1[:], in1=iotad[:], op=mybir.AluOpType.add)

                    et = e_tile if "s" not in ablate else None
                    if pending is not None:
                        consume(*pending)
                    pending = (t, kt, et)
                if pending is not None:
                    consume(*pending)
                    pending = None

            if "E" not in ablate:
                # ---- epilogue ----
                u = stage.tile([128, cols], F32, tag="u")
                nc.vector.tensor_scalar_add(u[:], kmax_all[:], MAG)
                lq = stage.tile([128, cols], F32, tag="lq")
                nc.vector.tensor_scalar_sub(lq[:], u[:], MAG)
                d = stage.tile([128, cols], F32, tag="d")
                nc.vector.tensor_sub(d[:], kmax_all[:], lq[:])
                acc = stage.tile([128, cols], F32, tag="acc")
                nc.vector.tensor_tensor(out=acc[:], in0=d[:], in1=labd[:], op=mybir.AluOpType.is_equal)

                em = stage.tile([128, cols], F32, tag="em")
                nc.scalar.activation(
                    em[:], lq[:], mybir.ActivationFunctionType.Exp, scale=(1.0 / SC)
                )
                rinv = stage.tile([128, cols], F32, tag="rinv")
                nc.vector.reciprocal(rinv[:], s_all[:])
                conf = stage.tile([128, cols], F32, tag="conf")
                nc.vector.tensor_mul(conf[:], em[:], rinv[:])

                y2 = stage.tile([128, cols], F32, tag="y2")
                nc.vector.tensor_scalar(
                    y2[:], conf[:], float(N_BINS), 0.5,
                    op0=mybir.AluOpType.mult, op1=mybir.AluOpType.subtract,
                )
                binb = stage.tile([128, cols], BF16, tag="binb")
                nc.vector.tensor_scalar(
                    binb[:], y2[:], MAG, MAG,
                    op0=mybir.AluOpType.add, op1=mybir.AluOpType.subtract,
                )

                # one-hot, group-contiguous layout: oh[p, g, b, j] so each
                # group's matmul lhsT slice is one contiguous 120-wide dim
                oh = stage.tile([128, N_BINS * cols], BF16, tag="oh")
                oh4 = oh[:].rearrange(
                    "p (g b j) -> p g b j", b=N_BINS, j=GROUPS_PER_MM
                )
                binb3 = binb[:].rearrange("p (g j) -> p g j", j=GROUPS_PER_MM)
                for b in range(N_BINS):
                    nc.vector.tensor_scalar(
                        oh4[:, :, b, :], binb3, float(b), None,
                        op0=mybir.AluOpType.is_equal,
                    )

                # V: interleaved (1, conf, acc) per staged column, bf16
                v_all = stage.tile([128, cols * 3], BF16, tag="v_all")
                nc.vector.memset(v_all[:], 1.0)
                v3 = v_all[:].rearrange("p (col three) -> p col three", three=3)
                nc.vector.tensor_copy(v3[:, :, 1], conf[:])
                nc.vector.tensor_copy(v3[:, :, 2], acc[:])

                # PE: accumulate per-bin partials over all column groups
                psum = psum_pool.tile([GROUPS_PER_MM * N_BINS, GROUPS_PER_MM * 3], F32)
                lw = N_BINS * GROUPS_PER_MM  # 120
                for g in range(n_groups):
                    nc.tensor.matmul(
                        out=psum[:],
                        lhsT=oh[:, g * lw : (g + 1) * lw],
                        rhs=v_all[:, g * GROUPS_PER_MM * 3 : (g + 1) * GROUPS_PER_MM * 3],
                        start=(g == 0),
                        stop=(g == n_groups - 1),
                    )

                out_sb = stage.tile([GROUPS_PER_MM * N_BINS, GROUPS_PER_MM * 3], F32, tag="out_sb")
                nc.vector.tensor_copy(out_sb[:], psum[:])
                nc.sync.dma_start(out=part[:], in_=out_sb[:])

    return nc


# ----------------------------------------------------------------------------
# host side
# ----------------------------------------------------------------------------

_RUNNER_CACHE = {}


def _get_runner(tiles=TILES, repeat=1, **opts):
    """Build (once) a jitted 8-core shard_map runner for the kernel."""
    key = (tiles, repeat, tuple(sorted(opts.items())))
    if key in _RUNNER_CACHE:
        return _RUNNER_CACHE[key]

    import jax
    from jax.sharding import Mesh, PartitionSpec
    try:
        from jax.experimental.shard_map import shard_map
    except ImportError:
        from jax.shard_map import shard_map
    from concourse import bass2jax

    nc = build_nc(tiles, repeat=repeat, **opts)
    bass2jax.install_neuronx_cc_hook()

    partition_name = nc.partition_id_tensor.name if nc.partition_id_tensor else None
    in_names = ["lg", "lb"]
    out_names = ["part"]
    out_avals = [jax.core.ShapedArray((GROUPS_PER_MM * N_BINS, GROUPS_PER_MM * 3), np.float32)]
    all_in_names = in_names + out_names + ([partition_name] if partition_name else [])

    def _body(*args):
        operands = list(args)
        if partition_name is not None:
            operands.append(bass2jax.partition_id_tensor())
        outs = bass2jax._bass_exec_p.bind(
            *operands,
            out_avals=tuple(out_avals),
            in_names=tuple(all_in_names),
            out_names=tuple(out_names),
            lowering_input_output_aliases=(),
            sim_require_finite=True,
            sim_require_nnan=True,
            nc=nc,
        )
        return tuple(outs)

    devices = jax.devices()[:N_CORES]
    mesh = Mesh(np.asarray(devices), ("core",))
    n_in = len(in_names) + len(out_avals)
    sharded = jax.jit(
        shard_map(
            _body,
            mesh=mesh,
            in_specs=(PartitionSpec("core"),) * n_in,
            out_specs=(PartitionSpec("core"),) * len(out_names),
            check_rep=False,
        ),
        donate_argnums=(len(in_names),),
        keep_unused=True,
    )
    _RUNNER_CACHE[key] = sharded
    return sharded


def _prep_inputs(logits, labels):
    logits = np.asarray(logits)
    labels = np.asarray(labels)
    n = logits.shape[0]
    assert logits.shape == (N_FULL, C) and n == N_FULL, logits.shape
    pad = N_PAD - n
    lg = np.concatenate([logits, np.zeros((pad, C), np.float32)], axis=0)
    lab32 = np.full(N_PAD, PAD_LABEL, np.int32)
    lab32[:n] = labels.astype(np.int32)
    # device layout: lab[core*128 + p, t*A + a] = label[core, t, p, a]
    lb = np.ascontiguousarray(
        lab32.reshape(N_CORES, TILES, 128, A).transpose(0, 2, 1, 3)
    ).reshape(N_CORES * 128, TILES * A)
    return lg, lb, pad


def _finish(parts, pad):
    """parts: (8, 120, 24) psum dumps -> ECE scalar (f32 [1]).

    psum row b*8+j holds bin-b sums for column-slot j; valid data are the
    j==j' diagonal blocks of the 8 (j') column triples."""
    agg = np.zeros((N_BINS, 3), np.float64)
    for c in range(parts.shape[0]):
        for j in range(GROUPS_PER_MM):
            for b in range(N_BINS):
                agg[b] += parts[c, b * GROUPS_PER_MM + j, j * 3 : (j + 1) * 3]
    counts = agg[:, 0].copy()
    conf_sums = agg[:, 1].copy()
    acc_sums = agg[:, 2].copy()
    # all-zero pad rows: conf = exp(0)/100 -> bin 0, acc = 0 (pred 99 != 55)
    counts[0] -= pad
    conf_sums[0] -= pad * float(np.float32(1.0) / np.float32(100.0))
    n = N_FULL
    prop = counts / n
    denom = np.maximum(counts, 1.0)
    avg_conf = conf_sums / denom
    avg_acc = acc_sums / denom
    per_bin = np.where(counts > 0, np.abs(avg_conf - avg_acc) * prop, 0.0)
    return np.array([per_bin.sum()], dtype=np.float32)


def kernel(logits, labels):
    lg, lb, pad = _prep_inputs(logits, labels)
    runner = _get_runner()
    zeros = np.zeros((N_CORES * GROUPS_PER_MM * N_BINS, GROUPS_PER_MM * 3), np.float32)
    last = None
    for attempt in range(3):
        try:
            (out,) = runner(lg, lb, np.zeros_like(zeros))
            parts = np.asarray(out).reshape(
                N_CORES, GROUPS_PER_MM * N_BINS, GROUPS_PER_MM * 3
            )
            return _finish(parts, pad)
        except Exception as e:  # transient NRT_EXEC_UNIT_UNRECOVERABLE etc.
            last = e
            import time as _time

            _time.sleep(20)
    raise last


# revision 30
# speedup vs baseline: 1.0660x; 1.0267x over previous
"""Expected Calibration Error kernel for 8 Trainium2 NeuronCores.

Design (v2, "quantized-grid tilt-pack"):
  - Pad N=1,000,000 rows to 1,007,616 = 8 * 123 * 1024 with all-zero logit
    rows (label 55); each core processes 123 tiles of 1024 rows
    ([128 partitions x (8 rows * 100 classes)]).
  - Per tile:
      ACT:  t = l*1024 + 1.5*2^23      (magic add: rounds l*1024 to int)
      DVE:  r = t - 1.5*2^23           (= round(l*1024), integer f32)
      ACT:  e = exp(t*2^-10 - 12288) -> bf16   (= exp(r/1024), quantized-
            logit softmax numerator; |l|<8 so no overflow)
      k-chain (argmax+max packed in one value, delta = 2^-10):
        kt   = r + iota100*delta                (GPSIMD tensor_tensor add)
        kmax = segmented max(kt)                (DVE tensor_reduce)
      => kmax = r_max + delta*argmax EXACTLY (r integer, tilt < 0.5, f32
         grid arithmetic exact for |r| < 2^13).
      S-chain: bf16 pairwise adds (2x DVE mode) + f32 reduce:
        sh1 = e_lo50 + e_hi50; sh2 = sh1_lo25 + sh1_hi25; S = sum(sh2)
      The per-tile reduces are issued two tiles late (software pipelining)
      so the in-order DVE never head-blocks on GPSIMD/ACT producers.
  - Epilogue (batched over 984 staged columns):
      lq = round(kmax) via magic; c* = (kmax - lq)*1024 exact;
      acc = (kmax - lq == label*delta); conf = exp(lq*2^-10) * (1/S);
      bin = round(15*conf - 0.5) via magic -> bf16;
      one-hot per bin via 15 tensor_scalar is_eq (bf16, 4x mode);
      PE matmuls accumulate per-bin (count, conf, acc) into PSUM [120, 24]
      (rows b*8+j; valid blocks on the j==j' diagonal).
  - Host: sum diagonal blocks over cores/j, subtract pad-row contribution
    (bin 0, conf 1/100, acc 0), finish the 15-bin ECE reduction.
"""

import sys

sys.path.insert(0, "/opt/trn_rl_repo")

import numpy as np

import concourse.bass as bass
import concourse.mybir as mybir
import concourse.tile as tile
from concourse.vector_clock import ScopedClock

F32 = mybir.dt.float32
I32 = mybir.dt.int32
BF16 = mybir.dt.bfloat16

N_BINS = 15
C = 100
ROWS_PER_TILE = 1024
A = 8  # rows per partition per tile
N_FULL = 1_000_000
N_CORES = 8
TILES = 123
ROWS_PER_CORE = TILES * ROWS_PER_TILE  # 125952
N_PAD = N_CORES * ROWS_PER_CORE       # 1007616
PAD_LABEL = 55
GROUPS_PER_MM = 8

MAG = 12582912.0          # 1.5 * 2^23: f32 magic for round-to-int
SC = 1024.0               # logit quantization scale (grid 2^-10)
DELTA = 1.0 / 1024.0      # class-index tilt unit (99*DELTA < 0.5)


def _patch_tile_drain():
    """walrus in this container allows only 1 sync wait per instruction; split
    every multi-wait instruction's extra waits onto preceding same-engine
    no-ops, and the TileContext exit drain's waits across a chain of drains."""
    if getattr(tile.TileContext, "_drain_patched", False):
        return

    orig_lower = tile.TileContext._lower_ordered_insts

    def _lower_ordered_insts(self, ordered):
        for insts in ordered.values():
            if not isinstance(insts, list):
                continue
            new = []
            for inst in insts:
                si = getattr(inst, "sync_info", None)
                waits = list(si.on_wait) if si is not None else []
                if len(waits) > 1 and isinstance(inst, mybir.Instruction):
                    si.on_wait = waits[-1:]
                    for j, w in enumerate(waits[:-1]):
                        nop = mybir.InstNoOp(
                            name=f"{inst.name}-ws{j}",
                            sync_info=mybir.SyncInfo(on_wait=[w], on_update=[]),
                            bass_nofuse=True,
                            engine=inst.engine,
                        )
                        new.append(nop)
                new.append(inst)
            insts[:] = new
        return orig_lower(self, ordered)

    tile.TileContext._lower_ordered_insts = _lower_ordered_insts

    orig_commit = tile.TileContext._commit_instruction

    def _commit_instruction(self, inst, *args, **kwargs):
        si = getattr(inst, "sync_info", None)
        if si is not None and len(si.on_wait) > 1 and isinstance(inst, mybir.Instruction):
            waits = list(si.on_wait)
            si.on_wait = waits[-1:]
            for j, w in enumerate(waits[:-1]):
                nop = mybir.InstNoOp(
                    name=f"{inst.name}-cs{j}",
                    sync_info=mybir.SyncInfo(on_wait=[w], on_update=[]),
                    bass_nofuse=True,
                    engine=inst.engine,
                )
                orig_commit(self, nop)
        return orig_commit(self, inst, *args, **kwargs)

    tile.TileContext._commit_instruction = _commit_instruction

    def _drain_and_barrier(self, tick_clock, wait_clock):
        drain_inst = self.nc.sync.drain()
        wait_clock.add_sem_waits(
            drain_inst.ins, ScopedClock({None: tick_clock.global_clock})
        )
        si = drain_inst.ins.sync_info
        waits = list(si.on_wait) if si else []
        if len(waits) > 1:
            si.on_wait = waits[:1]
            for i in range(1, len(waits)):
                d2 = self.nc.sync.drain()
                d2.ins.sync_info = type(si)(on_wait=waits[i : i + 1], on_update=[])
        self.nc.all_engine_barrier()
        popped = self.nc._tile_sem_poison_stack.pop()
        assert popped is self._sem_poison
        self.nc.clear_and_free_semaphores(list(self.sems.allocated().values()))
        self.nc.all_engine_barrier()

    tile.TileContext._drain_and_barrier = _drain_and_barrier
    tile.TileContext._drain_patched = True


def build_nc(tiles=TILES, repeat=1, ablate="", io_bufs=8, ek_bufs=4,
             kh_eng="dve", kt_eng="gpsimd", klevel=0, r_act_m=0, lag=2, sfirst=0):
    """ablate letters: 'q' quantize, 'k' k-chain, 'e' exp, 's' S-chain,
    'E' epilogue, 'D' per-tile input DMA.
    klevel: 0 = flat tilt-add (kt_eng) + full-width max-reduce;
            1 = one halving level (kh_eng max) before tilt+reduce."""
    _patch_tile_drain()
    cols = tiles * A
    n_groups = cols // GROUPS_PER_MM
    assert cols % GROUPS_PER_MM == 0

    nc = bass.Bass(trn_type="TRN2")
    lg = nc.declare_dram_parameter("lg", [tiles * ROWS_PER_TILE, C], F32, isOutput=False)
    lb = nc.declare_dram_parameter("lb", [128, cols], I32, isOutput=False)
    part = nc.declare_dram_parameter("part", [GROUPS_PER_MM * N_BINS, GROUPS_PER_MM * 3], F32, isOutput=True)

    khe = getattr(nc, {"gpsimd": "gpsimd", "dve": "vector"}[kh_eng])
    kte = getattr(nc, {"gpsimd": "gpsimd", "dve": "vector"}[kt_eng])

    with tile.TileContext(nc) as tc:
        with (
            tc.tile_pool(name="io", bufs=io_bufs) as io_pool,
            tc.tile_pool(name="ek", bufs=ek_bufs) as ek_pool,
            tc.tile_pool(name="stage", bufs=1) as stage,
            tc.tile_pool(name="psum", bufs=1, space="PSUM") as psum_pool,
        ):
            # ---- constants ----
            # tilt tiles: iota * DELTA over the within-segment class index
            tw = 50 if klevel == 1 else C  # tilt width per segment
            it_i = stage.tile([128, A * tw], I32, tag="it_i")
            nc.gpsimd.iota(it_i[:], pattern=[[0, A], [1, tw]], base=0, channel_multiplier=0)
            iotad = stage.tile([128, A * tw], F32, tag="iotad")
            nc.vector.tensor_copy(iotad[:], it_i[:])
            nc.vector.tensor_scalar_mul(iotad[:], iotad[:], DELTA)

            # per-partition scalar bias tiles for ACT
            bias_mag = stage.tile([128, 1], F32, tag="bias_mag")
            nc.vector.memset(bias_mag[:], MAG)
            bias_m12288 = stage.tile([128, 1], F32, tag="bias_m12288")
            nc.vector.memset(bias_m12288[:], -12288.0)

            # ---- staging ----
            kmax_all = stage.tile([128, cols], F32, tag="kmax_all")
            s_all = stage.tile([128, cols], F32, tag="s_all")
            lab_all = stage.tile([128, cols], I32, tag="lab_all")
            if ablate:
                nc.vector.memset(kmax_all[:], 0.0)
                nc.vector.memset(s_all[:], 1.0)

            # labels: one contiguous DMA (host pre-transposed layout)
            nc.sync.dma_start(out=lab_all[:], in_=lb[:, :])
            labd = stage.tile([128, cols], F32, tag="labd")
            nc.vector.tensor_copy(labd[:], lab_all[:])
            nc.vector.tensor_scalar_mul(labd[:], labd[:], DELTA)

            if "D" in ablate:
                fixed_tile = stage.tile([128, A * C], F32, tag="fixed_tile")
                nc.sync.dma_start(
                    out=fixed_tile[:],
                    in_=lg[0:ROWS_PER_TILE, :].rearrange("(p a) c -> p (a c)", p=128),
                )

            import contextlib

            loop_cm = tc.For_i(0, repeat, 1) if repeat > 1 else contextlib.nullcontext()

            def consume_s(t, e_t):
                if e_t is not None:
                    e3 = e_t[:].rearrange("p (a c) -> p a c", c=C)
                    sh1 = ek_pool.tile([128, A * 50], BF16, tag="sh1")
                    sh13 = sh1[:].rearrange("p (a h) -> p a h", h=50)
                    nc.vector.tensor_tensor(out=sh13, in0=e3[:, :, 0:50], in1=e3[:, :, 50:100], op=mybir.AluOpType.add)
                    sh2 = ek_pool.tile([128, A * 25], BF16, tag="sh2")
                    sh23 = sh2[:].rearrange("p (a h) -> p a h", h=25)
                    nc.vector.tensor_tensor(out=sh23, in0=sh13[:, :, 0:25], in1=sh13[:, :, 25:50], op=mybir.AluOpType.add)
                    nc.vector.tensor_reduce(
                        out=s_all[:, t * A : (t + 1) * A],
                        in_=sh23,
                        axis=mybir.AxisListType.X,
                        op=mybir.AluOpType.add,
                    )

            def consume(t, kt_t, e_t):
                """reduces for tile t (issued `lag` tiles late: producers done)."""
                if sfirst:
                    consume_s(t, e_t)
                    e_t = None
                if kt_t is not None:
                    h = C if klevel == 0 else 50
                    nc.vector.tensor_reduce(
                        out=kmax_all[:, t * A : (t + 1) * A],
                        in_=kt_t[:].rearrange("p (a h) -> p a h", h=h),
                        axis=mybir.AxisListType.X,
                        op=mybir.AluOpType.max,
                    )
                if e_t is not None:
                    e3 = e_t[:].rearrange("p (a c) -> p a c", c=C)
                    sh1 = ek_pool.tile([128, A * 50], BF16, tag="sh1")
                    sh13 = sh1[:].rearrange("p (a h) -> p a h", h=50)
                    nc.vector.tensor_tensor(out=sh13, in0=e3[:, :, 0:50], in1=e3[:, :, 50:100], op=mybir.AluOpType.add)
                    sh2 = ek_pool.tile([128, A * 25], BF16, tag="sh2")
                    sh23 = sh2[:].rearrange("p (a h) -> p a h", h=25)
                    nc.vector.tensor_tensor(out=sh23, in0=sh13[:, :, 0:25], in1=sh13[:, :, 25:50], op=mybir.AluOpType.add)
                    nc.vector.tensor_reduce(
                        out=s_all[:, t * A : (t + 1) * A],
                        in_=sh23,
                        axis=mybir.AxisListType.X,
                        op=mybir.AluOpType.add,
                    )

            with loop_cm:
                pendings = []  # (t, kt_tile, e_tile) awaiting reduces
                for t in range(tiles):
                    if "D" in ablate:
                        l_tile = fixed_tile
                    else:
                        l_tile = io_pool.tile([128, A * C], F32, tag="l_tile")
                        r0 = t * ROWS_PER_TILE
                        nc.sync.dma_start(
                            out=l_tile[:],
                            in_=lg[r0 : r0 + ROWS_PER_TILE, :].rearrange(
                                "(p a) c -> p (a c)", p=128
                            ),
                        )

                    if "q" in ablate:
                        t_tile = l_tile
                        r_tile = l_tile
                    else:
                        t_tile = ek_pool.tile([128, A * C], F32, tag="t_tile")
                        nc.scalar.activation(
                            t_tile[:], l_tile[:],
                            mybir.ActivationFunctionType.Identity,
                            bias=bias_mag[:], scale=SC,
                        )
                        r_tile = ek_pool.tile([128, A * C], F32, tag="r_tile")
                        if r_act_m and t % r_act_m == r_act_m - 1:
                            # rebalance: ACT has slack, DVE is the wall
                            nc.scalar.activation(
                                r_tile[:], t_tile[:],
                                mybir.ActivationFunctionType.Copy,
                                bias=-MAG,
                            )
                        else:
                            nc.vector.tensor_scalar_sub(r_tile[:], t_tile[:], MAG)

                    e_tile = None
                    if "e" not in ablate:
                        e_tile = ek_pool.tile([128, A * C], BF16, tag="e_tile")
                        nc.scalar.activation(
                            e_tile[:], t_tile[:],
                            mybir.ActivationFunctionType.Exp,
                            bias=bias_m12288[:], scale=(1.0 / SC),
                        )

                    kt = None
                    if "k" not in ablate:
                        r3 = r_tile[:].rearrange("p (a c) -> p a c", c=C)
                        if klevel == 0:
                            # flat: kt = r + iota*DELTA (GP add)
                            kt = ek_pool.tile([128, A * C], F32, tag="kt")
                            kte.tensor_tensor(out=kt[:], in0=r_tile[:], in1=iotad[:], op=mybir.AluOpType.add)
                        else:
                            # one halving level: z1 = hi + 50d; kh1 = max(lo, z1)
                            z1 = ek_pool.tile([128, A * 50], F32, tag="z1")
                            z13 = z1[:].rearrange("p (a h) -> p a h", h=50)
                            nc.vector.tensor_scalar_add(z13, r3[:, :, 50:100], 50.0 * DELTA)
                            kh1 = ek_pool.tile([128, A * 50], F32, tag="kh1")
                            kh13 = kh1[:].rearrange("p (a h) -> p a h", h=50)
                            khe.tensor_tensor(out=kh13, in0=r3[:, :, 0:50], in1=z13, op=mybir.AluOpType.max)
                            kt = ek_pool.tile([128, A * 50], F32, tag="kt")
                            kte.tensor_tensor(out=kt[:], in0=kh1[:], in1=iotad[:], op=mybir.AluOpType.add)

                    et = e_tile if "s" not in ablate else None
                    pendings.append((t, kt, et))
                    if len(pendings) > lag:
                        consume(*pendings.pop(0))
                for p in pendings:
                    consume(*p)
                pendings = []

            if "E" not in ablate:
                # ---- epilogue ----
                lq = stage.tile([128, cols], F32, tag="lq")
                nc.vector.tensor_scalar(
                    lq[:], kmax_all[:], MAG, MAG,
                    op0=mybir.AluOpType.add, op1=mybir.AluOpType.subtract,
                )
                d = stage.tile([128, cols], F32, tag="d")
                nc.vector.tensor_sub(d[:], kmax_all[:], lq[:])
                acc = stage.tile([128, cols], F32, tag="acc")
                nc.vector.tensor_tensor(out=acc[:], in0=d[:], in1=labd[:], op=mybir.AluOpType.is_equal)

                em = stage.tile([128, cols], F32, tag="em")
                nc.scalar.activation(
                    em[:], lq[:], mybir.ActivationFunctionType.Exp, scale=(1.0 / SC)
                )
                rinv = stage.tile([128, cols], F32, tag="rinv")
                nc.vector.reciprocal(rinv[:], s_all[:])
                conf = stage.tile([128, cols], F32, tag="conf")
                nc.vector.tensor_mul(conf[:], em[:], rinv[:])

                y2 = stage.tile([128, cols], F32, tag="y2")
                nc.vector.tensor_scalar(
                    y2[:], conf[:], float(N_BINS), 0.5,
                    op0=mybir.AluOpType.mult, op1=mybir.AluOpType.subtract,
                )
                binb = stage.tile([128, cols], BF16, tag="binb")
                nc.vector.tensor_scalar(
                    binb[:], y2[:], MAG, MAG,
                    op0=mybir.AluOpType.add, op1=mybir.AluOpType.subtract,
                )

                # one-hot, group-contiguous layout: oh[p, g, b, j] so each
                # group's matmul lhsT slice is one contiguous 120-wide dim
                oh = stage.tile([128, N_BINS * cols], BF16, tag="oh")
                oh4 = oh[:].rearrange(
                    "p (g b j) -> p g b j", b=N_BINS, j=GROUPS_PER_MM
                )
                binb3 = binb[:].rearrange("p (g j) -> p g j", j=GROUPS_PER_MM)
                for b in range(N_BINS):
                    nc.vector.tensor_scalar(
                        oh4[:, :, b, :], binb3, float(b), None,
                        op0=mybir.AluOpType.is_equal,
                    )

                # V: interleaved (1, conf, acc) per staged column, bf16
                v_all = stage.tile([128, cols * 3], BF16, tag="v_all")
                nc.vector.memset(v_all[:], 1.0)
                v3 = v_all[:].rearrange("p (col three) -> p col three", three=3)
                nc.vector.tensor_copy(v3[:, :, 1], conf[:])
                nc.vector.tensor_copy(v3[:, :, 2], acc[:])

                # PE: accumulate per-bin partials over all column groups
                psum = psum_pool.tile([GROUPS_PER_MM * N_BINS, GROUPS_PER_MM * 3], F32)
                lw = N_BINS * GROUPS_PER_MM  # 120
                for g in range(n_groups):
                    nc.tensor.matmul(
                        out=psum[:],
                        lhsT=oh[:, g * lw : (g + 1) * lw],
                        rhs=v_all[:, g * GROUPS_PER_MM * 3 : (g + 1) * GROUPS_PER_MM * 3],
                        start=(g == 0),
                        stop=(g == n_groups - 1),
                    )

                out_sb = stage.tile([GROUPS_PER_MM * N_BINS, GROUPS_PER_MM * 3], F32, tag="out_sb")
                nc.vector.tensor_copy(out_sb[:], psum[:])
                nc.sync.dma_start(out=part[:], in_=out_sb[:])

    return nc


# ----------------------------------------------------------------------------
# host side
# ----------------------------------------------------------------------------

_RUNNER_CACHE = {}


def _get_runner(tiles=TILES, repeat=1, **opts):
    """Build (once) a jitted 8-core shard_map runner for the kernel."""
    key = (tiles, repeat, tuple(sorted(opts.items())))
    if key in _RUNNER_CACHE:
        return _RUNNER_CACHE[key]

    import jax
    from jax.sharding import Mesh, PartitionSpec
    try:
        from jax.experimental.shard_map import shard_map
    except ImportError:
        from jax.shard_map import shard_map
    from concourse import bass2jax

    nc = build_nc(tiles, repeat=repeat, **opts)
    bass2jax.install_neuronx_cc_hook()

    partition_name = nc.partition_id_tensor.name if nc.partition_id_tensor else None
    in_names = ["lg", "lb"]
    out_names = ["part"]
    out_avals = [jax.core.ShapedArray((GROUPS_PER_MM * N_BINS, GROUPS_PER_MM * 3), np.float32)]
    all_in_names = in_names + out_names + ([partition_name] if partition_name else [])

    def _body(*args):
        operands = list(args)
        if partition_name is not None:
            operands.append(bass2jax.partition_id_tensor())
        outs = bass2jax._bass_exec_p.bind(
            *operands,
            out_avals=tuple(out_avals),
            in_names=tuple(all_in_names),
            out_names=tuple(out_names),
            lowering_input_output_aliases=(),
            sim_require_finite=True,
            sim_require_nnan=True,
            nc=nc,
        )
        return tuple(outs)

    devices = jax.devices()[:N_CORES]
    mesh = Mesh(np.asarray(devices), ("core",))
    n_in = len(in_names) + len(out_avals)
    sharded = jax.jit(
        shard_map(
            _body,
            mesh=mesh,
            in_specs=(PartitionSpec("core"),) * n_in,
            out_specs=(PartitionSpec("core"),) * len(out_names),
            check_rep=False,
        ),
        donate_argnums=(len(in_names),),
        keep_unused=True,
    )
    _RUNNER_CACHE[key] = sharded
    return sharded


def _prep_inputs(logits, labels):
    logits = np.asarray(logits)
    labels = np.asarray(labels)
    n = logits.shape[0]
    assert logits.shape == (N_FULL, C) and n == N_FULL, logits.shape
    pad = N_PAD - n
    lg = np.concatenate([logits, np.zeros((pad, C), np.float32)], axis=0)
    lab32 = np.full(N_PAD, PAD_LABEL, np.int32)
    lab32[:n] = labels.astype(np.int32)
    # device layout: lab[core*128 + p, t*A + a] = label[core, t, p, a]
    lb = np.ascontiguousarray(
        lab32.reshape(N_CORES, TILES, 128, A).transpose(0, 2, 1, 3)
    ).reshape(N_CORES * 128, TILES * A)
    return lg, lb, pad


def _finish(parts, pad):
    """parts: (8, 120, 24) psum dumps -> ECE scalar (f32 [1]).

    psum row b*8+j holds bin-b sums for column-slot j; valid data are the
    j==j' diagonal blocks of the 8 (j') column triples."""
    agg = np.zeros((N_BINS, 3), np.float64)
    for c in range(parts.shape[0]):
        for j in range(GROUPS_PER_MM):
            for b in range(N_BINS):
                agg[b] += parts[c, b * GROUPS_PER_MM + j, j * 3 : (j + 1) * 3]
    counts = agg[:, 0].copy()
    conf_sums = agg[:, 1].copy()
    acc_sums = agg[:, 2].copy()
    # all-zero pad rows: conf = exp(0)/100 -> bin 0, acc = 0 (pred 99 != 55)
    counts[0] -= pad
    conf_sums[0] -= pad * float(np.float32(1.0) / np.float32(100.0))
    n = N_FULL
    prop = counts / n
    denom = np.maximum(counts, 1.0)
    avg_conf = conf_sums / denom
    avg_acc = acc_sums / denom
    per_bin = np.where(counts > 0, np.abs(avg_conf - avg_acc) * prop, 0.0)
    return np.array([per_bin.sum()], dtype=np.float32)


def kernel(logits, labels):
    lg, lb, pad = _prep_inputs(logits, labels)
    runner = _get_runner()
    zeros = np.zeros((N_CORES * GROUPS_PER_MM * N_BINS, GROUPS_PER_MM * 3), np.float32)
    last = None
    for attempt in range(3):
        try:
            (out,) = runner(lg, lb, np.zeros_like(zeros))
            parts = np.asarray(out).reshape(
                N_CORES, GROUPS_PER_MM * N_BINS, GROUPS_PER_MM * 3
            )
            return _finish(parts, pad)
        except Exception as e:  # transient NRT_EXEC_UNIT_UNRECOVERABLE etc.
            last = e
            import time as _time

            _time.sleep(20)
    raise last
